# revision 19
# baseline (speedup 1.0000x reference)
"""Multi-head causal attention (B=4, T=2048, D=1024, H=16, d_k=64) on 8 trn2 cores.

Sharding (Megatron-style): core c handles batch b=c//2 and head-group
g=c%2 (8 heads). Each core computes its batch's QKV projection restricted
to its head group, causal attention for those heads, and the partial
output projection. Host sums the two head-group partials per batch.

Per-core layouts (host pre-arranges; all fp32):
  xT  [1024, 2048]  = x[b].T           (d_model on rows)
  wq/wk/wv [1024, 512]                 (columns of w_qkv for the group)
  wo  [512, 1024]                      (rows of w_out for the group)
  outT [1024, 2048] = partial out.T

On-chip dataflow (all matmuls fp32r = fp22 mantissa, full PE rate):
  V    = xT_chunk.T @ wv         -> V_sb [k, d] with a ones column per head
  Q.T  = wq_chunk.T @ xT         -> QT [d, q] (head pairs on 128 partitions)
  K.T  likewise                  -> KT [d, q]
  per q-chunk (outer), head-pair (inner), k-tile pairs:
    scoresT[k, q]  = KT_tile.T @ QT_tile  (two heads row-packed, K=64)
                     pairs of k-tiles share a [128,1024] 2-bank PSUM tile
    expT = exp(0.125 * scoresT)  (one ACT op per [128,1024] pair)
    causal mask on diagonal pairs (DVE/GpSimd split)
    AV.T[d|den, q] += (V|1).T @ expT      (PSUM accumulate)
  per q-chunk epilogue (overlaps next q-chunk's attention):
    batched reciprocal of denominators, indicator-matmul broadcast,
    in-place divide, then the out-projection for that q-chunk.
"""
import numpy as np
from contextlib import ExitStack

import concourse.bass as bass
import concourse.mybir as mybir
import concourse.tile as tile
from concourse import bacc
from concourse.bass_utils import run_bass_kernel_spmd

P = 128
T = 2048           # tokens per core (one batch)
D = 1024           # d_model
DG = 512           # head-group width = 8 heads * 64
DK = 64
QC = 512           # q-chunk (matmul moving free dim)
NQC = T // QC      # 4
NKO = T // P       # 16 k-tiles
NEO = D // P       # 8 e-tiles (contraction d_model)
NDO = DG // P      # 4 head-pair chunks
f32 = mybir.dt.float32
f32r = mybir.dt.float32r
AF = mybir.ActivationFunctionType
ALU = mybir.AluOpType
SCALE = 0.125      # 1/sqrt(d_k)

_CACHE: dict = {}

_EYE8 = np.eye(8, dtype=np.float32)
_IND4 = np.zeros((NDO, 8, P), dtype=np.float32)
for _do in range(NDO):
    _IND4[_do, 2 * _do, 0:64] = 1.0
    _IND4[_do, 2 * _do + 1, 64:128] = 1.0


def _build():
    nc = bacc.Bacc("TRN2", target_bir_lowering=False, debug=False)
    xT = nc.dram_tensor("xT", (D, T), f32, kind="ExternalInput").ap()
    wq = nc.dram_tensor("wq", (D, DG), f32, kind="ExternalInput").ap()
    wk = nc.dram_tensor("wk", (D, DG), f32, kind="ExternalInput").ap()
    wv = nc.dram_tensor("wv", (D, DG), f32, kind="ExternalInput").ap()
    wo = nc.dram_tensor("wo", (DG, D), f32, kind="ExternalInput").ap()
    eye8 = nc.dram_tensor("eye8", (8, 8), f32, kind="ExternalInput").ap()
    ind4 = nc.dram_tensor("ind4", (NDO, 8, P), f32, kind="ExternalInput").ap()
    outT = nc.dram_tensor("outT", (D, T), f32, kind="ExternalOutput").ap()

    with tile.TileContext(nc) as tc, ExitStack() as ctx:
        const = ctx.enter_context(tc.tile_pool(name="const", bufs=1))
        qkv_sb = ctx.enter_context(tc.tile_pool(name="qkv", bufs=1))

        # causal masks for diagonal k-tiles, duplicated for both heads:
        # cmask[j][:, s*512:(s+1)*512] keeps qi - ki - 128*j >= 0 (s = head A/B)
        cmask = []
        for j in range(4):
            m = const.tile([P, 2 * QC], f32, tag=f"cmask{j}", name=f"cmask{j}")
            nc.gpsimd.memset(m[:], 1.0)
            for s in range(2):
                nc.gpsimd.affine_select(
                    out=m[:, s * QC:(s + 1) * QC], in_=m[:, s * QC:(s + 1) * QC],
                    compare_op=ALU.is_ge, fill=0.0,
                    base=-128 * j, pattern=[[1, QC]],
                    channel_multiplier=-1,
                )
            cmask.append(m)
        ones8 = const.tile([P, 8], f32, tag="ones8")
        nc.gpsimd.memset(ones8[:], 1.0)
        # one-hot rows for scattering denominators onto PSUM partitions,
        # and per-do indicators for broadcasting reciprocals back out
        eyes = []
        for i in range(8):
            e = const.tile([1, 8], f32r, tag=f"eye{i}", name=f"eye{i}")
            nc.sync.dma_start(e[:], eye8[i:i + 1, :].bitcast(f32r))
            eyes.append(e)
        ind_r = []
        for do in range(NDO):
            ir = const.tile([8, P], f32r, tag=f"ind_r{do}", name=f"ind_r{do}")
            nc.sync.dma_start(ir[:], ind4[do].bitcast(f32r))
            ind_r.append(ir)

        # ---- Phase B: projections ----
        QT = [qkv_sb.tile([P, T], f32r, tag=f"QT{do}", name=f"QT{do}")
              for do in range(NDO)]
        KT = [qkv_sb.tile([P, T], f32r, tag=f"KT{do}", name=f"KT{do}")
              for do in range(NDO)]
        V = [qkv_sb.tile([P, 8 * 65], f32r, tag=f"V{ko}", name=f"V{ko}")
             for ko in range(NKO)]

        with tc.tile_pool(name="xTp", bufs=1) as xT_pool, \
             tc.tile_pool(name="wvp", bufs=1) as wv_pool, \
             tc.tile_pool(name="wqk", bufs=1) as wqk_pool, \
             tc.tile_pool(name="expp", bufs=3) as expp, \
             tc.tile_pool(name="dens", bufs=1) as dens, \
             tc.tile_pool(name="outp", bufs=3) as out_pool, \
             tc.tile_pool(name="attnp", bufs=1) as attn_sb, \
             tc.tile_pool(name="wop", bufs=1) as wo_pool, \
             tc.tile_pool(name="pj", bufs=2, space="PSUM") as ps_j, \
             tc.tile_pool(name="ps2p", bufs=2, space="PSUM") as ps_s, \
             tc.tile_pool(name="avp", bufs=1, space="PSUM") as ps_av:
            xTs = []
            for eo in range(NEO):
                xt = xT_pool.tile([P, T], f32r, tag=f"xT{eo}", name=f"xT{eo}")
                nc.sync.dma_start(xt[:], xT[eo * P:(eo + 1) * P, :].bitcast(f32r))
                xTs.append(xt)
            wvs = []
            for eo in range(NEO):
                wvt = wv_pool.tile([P, DG], f32r, tag=f"wv{eo}", name=f"wv{eo}")
                nc.sync.dma_start(wvt[:], wv[eo * P:(eo + 1) * P, :].bitcast(f32r))
                wvs.append(wvt)
            wos = []
            for do in range(NDO):
                wot = wo_pool.tile([P, D], f32r, tag=f"wo{do}", name=f"wo{do}")
                nc.sync.dma_start(wot[:], wo[do * P:(do + 1) * P, :].bitcast(f32r))
                wos.append(wot)
            ATT = [attn_sb.tile([P, T], f32r, tag=f"ATT{do}", name=f"ATT{do}")
                   for do in range(NDO)]

            # V projection: psv[k, d] = sum_e xT[e, k].T @ wv[e, d]
            for ko in range(NKO):
                psv = ps_j.tile([P, DG], f32, tag="pj", name=f"psv{ko}")
                ks = slice(ko * P, (ko + 1) * P)
                for eo in range(NEO):
                    nc.tensor.matmul(psv[:], xTs[eo][:, ks], wvs[eo][:],
                                     start=(eo == 0), stop=(eo == NEO - 1))
                dst = V[ko][:].rearrange("p (h c) -> p h c", c=65)
                srcv = psv[:].rearrange("p (h c) -> p h c", c=DK)
                nc.vector.tensor_copy(dst[:, :, 0:DK], srcv)
                nc.vector.tensor_copy(dst[:, :, DK], ones8[:])

            # Q.T / K.T projection: ps[d, q] = sum_e w[e, d].T @ xT[e, q]
            for do in range(NDO):
                for w_ap, dstT, wtag in ((wq, QT, "q"), (wk, KT, "k")):
                    wts = []
                    for eo in range(NEO):
                        wt = wqk_pool.tile([P, P], f32r, tag=f"w{eo}",
                                           name=f"w{wtag}{do}_{eo}")
                        nc.sync.dma_start(
                            wt[:],
                            w_ap[eo * P:(eo + 1) * P,
                                 do * P:(do + 1) * P].bitcast(f32r))
                        wts.append(wt)
                    for qc in range(NQC):
                        ps = ps_j.tile([P, QC], f32, tag="pj",
                                       name=f"psqk_{wtag}{do}_{qc}")
                        for eo in range(NEO):
                            nc.tensor.matmul(
                                ps[:], wts[eo][:],
                                xTs[eo][:, qc * QC:(qc + 1) * QC],
                                start=(eo == 0), stop=(eo == NEO - 1))
                        nc.vector.tensor_copy(
                            dstT[do][:, qc * QC:(qc + 1) * QC], ps[:])

            # attention + epilogue + output projection, q-chunk outer
            for qc in range(NQC):
                qs = slice(qc * QC, (qc + 1) * QC)
                denf = dens.tile([1, 8 * QC], f32r, tag="denf", name=f"denf{qc}")
                nko = 4 * (qc + 1)
                for do in range(NDO):
                    hA, hB = 2 * do, 2 * do + 1
                    pav = ps_av.tile([65, QC], f32, tag="pavA",
                                     name=f"pav{qc}_{do}")
                    pbv = ps_av.tile([65, QC], f32, tag="pavB",
                                     name=f"pbv{qc}_{do}")
                    for ko in range(nko):
                        ks = slice(ko * P, (ko + 1) * P)
                        ps2 = ps_s.tile([P, 2 * QC], f32, tag="ps2",
                                        name=f"ps2_{qc}_{do}_{ko}")
                        nc.tensor.matmul(ps2[:, 0:QC], KT[do][0:64, ks],
                                         QT[do][0:64, qs],
                                         start=True, stop=True)
                        nc.tensor.matmul(ps2[:, QC:2 * QC], KT[do][64:128, ks],
                                         QT[do][64:128, qs],
                                         start=True, stop=True)
                        e2 = expp.tile([P, 2 * QC], f32r, tag="e2",
                                       name=f"e2_{qc}_{do}_{ko}")
                        nc.scalar.activation(e2[:], ps2[:], AF.Exp, scale=SCALE)
                        j = ko - 4 * qc
                        if j >= 0:
                            nc.vector.tensor_tensor(e2[:], e2[:], cmask[j][:],
                                                    ALU.mult)
                        nc.tensor.matmul(pav[:], V[ko][:, hA * 65:hA * 65 + 65],
                                         e2[:, 0:QC], start=(ko == 0),
                                         stop=(ko == nko - 1))
                        nc.tensor.matmul(pbv[:], V[ko][:, hB * 65:hB * 65 + 65],
                                         e2[:, QC:2 * QC], start=(ko == 0),
                                         stop=(ko == nko - 1))
                    nc.vector.tensor_copy(ATT[do][0:64, qs], pav[0:64, :])
                    nc.vector.tensor_copy(ATT[do][64:128, qs], pbv[0:64, :])
                    nc.vector.tensor_copy(
                        denf[0:1, (2 * do) * QC:(2 * do + 1) * QC], pav[64:65, :])
                    nc.vector.tensor_copy(
                        denf[0:1, (2 * do + 1) * QC:(2 * do + 2) * QC],
                        pbv[64:65, :])
                # epilogue: scatter denominators to 8 PSUM partitions,
                # one batched reciprocal, broadcast back, in-place divide
                d8 = ps_j.tile([P, QC], f32, tag="pj", name=f"d8{qc}")
                for i in range(8):
                    nc.tensor.matmul(d8[0:8, :], eyes[i][:],
                                     denf[0:1, i * QC:(i + 1) * QC],
                                     start=(i == 0), stop=(i == 7))
                rec8 = dens.tile([8, QC], f32r, tag="rec8", name=f"rec8{qc}")
                with nc.allow_low_precision(
                        reason="fp32r rounding of softmax reciprocal is benign"):
                    nc.vector.reciprocal(rec8[:], d8[0:8, :])
                for do in range(NDO):
                    bc = ps_j.tile([P, QC], f32, tag="pj", name=f"bc{qc}_{do}")
                    nc.tensor.matmul(bc[:], ind_r[do][:], rec8[:],
                                     start=True, stop=True)
                    nc.vector.tensor_tensor(ATT[do][:, qs], ATT[do][:, qs],
                                            bc[:], ALU.mult)
                # output projection for this q-chunk
                for ec in range(NEO):
                    es = slice(ec * P, (ec + 1) * P)
                    pso = ps_j.tile([P, QC], f32, tag="pj",
                                    name=f"pso{qc}_{ec}")
                    for do in range(NDO):
                        nc.tensor.matmul(pso[:], wos[do][:, es],
                                         ATT[do][:, qs],
                                         start=(do == 0), stop=(do == NDO - 1))
                    osb = out_pool.tile([P, QC], f32, tag="osb",
                                        name=f"osb{qc}_{ec}")
                    nc.vector.tensor_copy(osb[:], pso[:])
                    nc.sync.dma_start(outT[es, qs], osb[:])
    nc.compile()
    return nc


def _get_nc():
    if "nc" not in _CACHE:
        _CACHE["nc"] = _build()
    return _CACHE["nc"]


def _in_maps(x, w_qkv, w_out):
    maps = []
    for c in range(8):
        b, g = c // 2, c % 2
        gs = slice(g * DG, (g + 1) * DG)
        maps.append({
            "xT": np.ascontiguousarray(x[b].T),
            "wq": np.ascontiguousarray(w_qkv[:, 0 * D:1 * D][:, gs]),
            "wk": np.ascontiguousarray(w_qkv[:, 1 * D:2 * D][:, gs]),
            "wv": np.ascontiguousarray(w_qkv[:, 2 * D:3 * D][:, gs]),
            "wo": np.ascontiguousarray(w_out[gs, :]),
            "eye8": _EYE8,
            "ind4": _IND4,
        })
    return maps


def kernel(x: np.ndarray, w_qkv: np.ndarray, w_out: np.ndarray) -> np.ndarray:
    x = np.asarray(x, dtype=np.float32)
    w_qkv = np.asarray(w_qkv, dtype=np.float32)
    w_out = np.asarray(w_out, dtype=np.float32)
    B = x.shape[0]
    nc = _get_nc()
    res = run_bass_kernel_spmd(nc, _in_maps(x, w_qkv, w_out),
                               core_ids=list(range(8)))
    out = np.empty((B, T, D), dtype=np.float32)
    for b in range(B):
        acc = res.results[2 * b]["outT"] + res.results[2 * b + 1]["outT"]
        out[b] = acc.T
    return out


# revision 20
# speedup vs baseline: 1.2238x; 1.2238x over previous
"""Multi-head causal attention (B=4, T=2048, D=1024, H=16, d_k=64) on 8 trn2 cores.

Sharding (Megatron-style): core c handles batch b=c//2 and head-group
g=c%2 (8 heads). Each core computes its batch's QKV projection restricted
to its head group, causal attention for those heads, and the partial
output projection. Host sums the two head-group partials per batch.

Per-core layouts (host pre-arranges; all fp32):
  xT  [1024, 2048]  = x[b].T           (d_model on rows)
  wq/wk/wv [1024, 512]                 (columns of w_qkv for the group)
  wo  [512, 1024]                      (rows of w_out for the group)
  outT [1024, 2048] = partial out.T

On-chip dataflow (all matmuls fp32r = fp22 mantissa, full PE rate):
  V    = xT_chunk.T @ wv         -> V_sb [k, d] with a ones column per head
  Q.T  = wq_chunk.T @ xT         -> QT [d, q] (head pairs on 128 partitions)
  K.T  likewise                  -> KT [d, q]
  per q-chunk (outer), head-pair, k-tile: one [128,1024] PSUM tile holds
  [scores_A | scores_B]; one ACT exp op covers both heads; causal mask via
  a free-dim-broadcast multiply on diagonal tiles; AV accumulates in PSUM
  with a ones column producing softmax denominators for free.
  Epilogue per q-chunk: denominators -> one flat tile -> one-hot scatter
  matmuls -> one batched reciprocal -> indicator-matmul broadcast ->
  in-place divide -> output projection (overlaps next q-chunk).
"""
import numpy as np
from contextlib import ExitStack

import concourse.bass as bass
import concourse.mybir as mybir
import concourse.tile as tile
from concourse import bacc
from concourse.bass_utils import run_bass_kernel_spmd

P = 128
T = 2048
D = 1024
DG = 512
DK = 64
QC = 512
NQC = T // QC      # 4
NKO = T // P       # 16
NEO = D // P       # 8
NDO = DG // P      # 4
f32 = mybir.dt.float32
f32r = mybir.dt.float32r
AF = mybir.ActivationFunctionType
ALU = mybir.AluOpType
SCALE = 0.125

_CACHE: dict = {}

_EYE8 = np.eye(8, dtype=np.float32)
_IND4 = np.zeros((NDO, 8, P), dtype=np.float32)
for _do in range(NDO):
    _IND4[_do, 2 * _do, 0:64] = 1.0
    _IND4[_do, 2 * _do + 1, 64:128] = 1.0


def _build():
    nc = bacc.Bacc("TRN2", target_bir_lowering=False, debug=False)
    xT = nc.dram_tensor("xT", (D, T), f32, kind="ExternalInput").ap()
    wq = nc.dram_tensor("wq", (D, DG), f32, kind="ExternalInput").ap()
    wk = nc.dram_tensor("wk", (D, DG), f32, kind="ExternalInput").ap()
    wv = nc.dram_tensor("wv", (D, DG), f32, kind="ExternalInput").ap()
    wo = nc.dram_tensor("wo", (DG, D), f32, kind="ExternalInput").ap()
    eye8 = nc.dram_tensor("eye8", (8, 8), f32, kind="ExternalInput").ap()
    ind4 = nc.dram_tensor("ind4", (NDO, 8, P), f32, kind="ExternalInput").ap()
    outT = nc.dram_tensor("outT", (D, T), f32, kind="ExternalOutput").ap()

    with tile.TileContext(nc) as tc, ExitStack() as ctx:
        const = ctx.enter_context(tc.tile_pool(name="const", bufs=1))
        qkv_sb = ctx.enter_context(tc.tile_pool(name="qkv", bufs=1))

        # causal masks for diagonal k-tiles (applied to both heads via a
        # free-dim broadcast): keep where qi - ki - 128*j >= 0
        cmask = []
        for j in range(4):
            m = const.tile([P, QC], f32, tag=f"cmask{j}", name=f"cmask{j}")
            nc.gpsimd.memset(m[:], 1.0)
            nc.gpsimd.affine_select(
                out=m[:], in_=m[:], compare_op=ALU.is_ge, fill=0.0,
                base=-128 * j, pattern=[[1, QC]], channel_multiplier=-1)
            cmask.append(m)
        ones8 = const.tile([P, 8], f32, tag="ones8")
        nc.gpsimd.memset(ones8[:], 1.0)
        eyes = []
        for i in range(8):
            e = const.tile([1, 8], f32r, tag=f"eye{i}", name=f"eye{i}")
            nc.sync.dma_start(e[:], eye8[i:i + 1, :].bitcast(f32r))
            eyes.append(e)
        ind_r = []
        for do in range(NDO):
            ir = const.tile([8, P], f32r, tag=f"ind_r{do}", name=f"ind_r{do}")
            nc.sync.dma_start(ir[:], ind4[do].bitcast(f32r))
            ind_r.append(ir)

        # ---- Phase B: projections ----
        QT = [qkv_sb.tile([P, T], f32r, tag=f"QT{do}", name=f"QT{do}")
              for do in range(NDO)]
        KT = [qkv_sb.tile([P, T], f32r, tag=f"KT{do}", name=f"KT{do}")
              for do in range(NDO)]
        V = [qkv_sb.tile([P, 8 * 65], f32r, tag=f"V{ko}", name=f"V{ko}")
             for ko in range(NKO)]

        with tc.tile_pool(name="xTp", bufs=1) as xT_pool, \
             tc.tile_pool(name="wvp", bufs=1) as wv_pool, \
             tc.tile_pool(name="wqk", bufs=10) as wqk_pool, \
             tc.tile_pool(name="psv", bufs=2, space="PSUM") as psv_pool, \
             tc.tile_pool(name="psqk", bufs=1, space="PSUM") as psqk_pool:
            xTs = []
            for eo in range(NEO):
                xt = xT_pool.tile([P, T], f32r, tag=f"xT{eo}", name=f"xT{eo}")
                nc.sync.dma_start(xt[:], xT[eo * P:(eo + 1) * P, :].bitcast(f32r))
                xTs.append(xt)
            wvs = []
            for eo in range(NEO):
                wvt = wv_pool.tile([P, DG], f32r, tag=f"wv{eo}", name=f"wv{eo}")
                nc.sync.dma_start(wvt[:], wv[eo * P:(eo + 1) * P, :].bitcast(f32r))
                wvs.append(wvt)

            # V projection: psv[k, d] = sum_e xT[e, k].T @ wv[e, d]
            for ko in range(NKO):
                psv = psv_pool.tile([P, DG], f32, tag="psv", name=f"psv{ko}")
                ks = slice(ko * P, (ko + 1) * P)
                for eo in range(NEO):
                    nc.tensor.matmul(psv[:], xTs[eo][:, ks], wvs[eo][:],
                                     start=(eo == 0), stop=(eo == NEO - 1))
                dst = V[ko][:].rearrange("p (h c) -> p h c", c=65)
                srcv = psv[:].rearrange("p (h c) -> p h c", c=DK)
                nc.vector.tensor_copy(dst[:, :, 0:DK], srcv)
                nc.vector.tensor_copy(dst[:, :, DK], ones8[:])

            # Q.T / K.T projection: ps[d, q] = sum_e w[e, d].T @ xT[e, q]
            for do in range(NDO):
                for w_ap, dstT, wtag in ((wq, QT, "q"), (wk, KT, "k")):
                    ps = [psqk_pool.tile([P, QC], f32, tag=f"psqk{qc}",
                                         name=f"psqk_{wtag}{do}_{qc}")
                          for qc in range(NQC)]
                    for eo in range(NEO):
                        wt = wqk_pool.tile([P, P], f32r, tag="w",
                                           name=f"w{wtag}{do}_{eo}")
                        nc.sync.dma_start(
                            wt[:],
                            w_ap[eo * P:(eo + 1) * P,
                                 do * P:(do + 1) * P].bitcast(f32r))
                        for qc in range(NQC):
                            nc.tensor.matmul(
                                ps[qc][:], wt[:],
                                xTs[eo][:, qc * QC:(qc + 1) * QC],
                                start=(eo == 0), stop=(eo == NEO - 1))
                    for qc in range(NQC):
                        nc.vector.tensor_copy(
                            dstT[do][:, qc * QC:(qc + 1) * QC], ps[qc][:])

        # ---- Phase C+D: attention + output projection, q-chunk outer ----
        attn_sb = ctx.enter_context(tc.tile_pool(name="attn", bufs=1))
        ATT = [attn_sb.tile([P, T], f32r, tag=f"ATT{do}", name=f"ATT{do}")
               for do in range(NDO)]
        wo_pool = ctx.enter_context(tc.tile_pool(name="wop", bufs=1))
        wos = []
        for do in range(NDO):
            wot = wo_pool.tile([P, D], f32r, tag=f"wo{do}", name=f"wo{do}")
            nc.sync.dma_start(wot[:], wo[do * P:(do + 1) * P, :].bitcast(f32r))
            wos.append(wot)

        with tc.tile_pool(name="expp", bufs=3) as expp, \
             tc.tile_pool(name="dens", bufs=1) as dens, \
             tc.tile_pool(name="outp", bufs=3) as out_pool, \
             tc.tile_pool(name="ps_s", bufs=2, space="PSUM") as ps_s, \
             tc.tile_pool(name="ps_av", bufs=1, space="PSUM") as ps_av, \
             tc.tile_pool(name="ps_bc", bufs=1, space="PSUM") as ps_bc, \
             tc.tile_pool(name="ps_o", bufs=1, space="PSUM") as ps_o:
            for qc in range(NQC):
                qs = slice(qc * QC, (qc + 1) * QC)
                denf = dens.tile([1, 8 * QC], f32r, tag="denf", name=f"denf{qc}")
                nko = 4 * (qc + 1)
                for do in range(NDO):
                    hA, hB = 2 * do, 2 * do + 1
                    pav = ps_av.tile([65, QC], f32, tag="pavA",
                                     name=f"pav{qc}_{do}")
                    pbv = ps_av.tile([65, QC], f32, tag="pavB",
                                     name=f"pbv{qc}_{do}")
                    for ko in range(nko):
                        ks = slice(ko * P, (ko + 1) * P)
                        ps2 = ps_s.tile([P, 2 * QC], f32, tag="ps2",
                                        name=f"ps2_{qc}_{do}_{ko}")
                        nc.tensor.matmul(ps2[:, 0:QC], KT[do][0:64, ks],
                                         QT[do][0:64, qs],
                                         start=True, stop=True)
                        nc.tensor.matmul(ps2[:, QC:2 * QC], KT[do][64:128, ks],
                                         QT[do][64:128, qs],
                                         start=True, stop=True)
                        e2 = expp.tile([P, 2 * QC], f32r, tag="e2",
                                       name=f"e2_{qc}_{do}_{ko}")
                        nc.scalar.activation(e2[:], ps2[:], AF.Exp, scale=SCALE)
                        j = ko - 4 * qc
                        if j >= 0:
                            cm = cmask[j][:, None, :].to_broadcast((P, 2, QC))
                            e2v = e2[:].rearrange("p (s n) -> p s n", s=2)
                            nc.vector.tensor_tensor(e2v, e2v, cm, ALU.mult)
                        nc.tensor.matmul(pav[:], V[ko][:, hA * 65:hA * 65 + 65],
                                         e2[:, 0:QC], start=(ko == 0),
                                         stop=(ko == nko - 1))
                        nc.tensor.matmul(pbv[:], V[ko][:, hB * 65:hB * 65 + 65],
                                         e2[:, QC:2 * QC], start=(ko == 0),
                                         stop=(ko == nko - 1))
                    nc.vector.tensor_copy(ATT[do][0:64, qs], pav[0:64, :])
                    nc.vector.tensor_copy(ATT[do][64:128, qs], pbv[0:64, :])
                    nc.vector.tensor_copy(
                        denf[0:1, (2 * do) * QC:(2 * do + 1) * QC], pav[64:65, :])
                    nc.vector.tensor_copy(
                        denf[0:1, (2 * do + 1) * QC:(2 * do + 2) * QC],
                        pbv[64:65, :])
                # epilogue: scatter denominators to 8 PSUM partitions,
                # one batched reciprocal, broadcast back, in-place divide
                d8 = ps_bc.tile([8, QC], f32, tag="d8", name=f"d8{qc}")
                for i in range(8):
                    nc.tensor.matmul(d8[:], eyes[i][:],
                                     denf[0:1, i * QC:(i + 1) * QC],
                                     start=(i == 0), stop=(i == 7))
                rec8 = dens.tile([8, QC], f32r, tag="rec8", name=f"rec8{qc}")
                with nc.allow_low_precision(
                        reason="fp32r rounding of softmax reciprocal is benign"):
                    nc.vector.reciprocal(rec8[:], d8[:])
                for do in range(NDO):
                    bc = ps_o.tile([P, QC], f32, tag="pso", name=f"bc{qc}_{do}")
                    nc.tensor.matmul(bc[:], ind_r[do][:], rec8[:],
                                     start=True, stop=True)
                    nc.vector.tensor_tensor(ATT[do][:, qs], ATT[do][:, qs],
                                            bc[:], ALU.mult)
                # output projection for this q-chunk
                for ec in range(NEO):
                    es = slice(ec * P, (ec + 1) * P)
                    pso = ps_o.tile([P, QC], f32, tag="pso",
                                    name=f"pso{qc}_{ec}")
                    for do in range(NDO):
                        nc.tensor.matmul(pso[:], wos[do][:, es],
                                         ATT[do][:, qs],
                                         start=(do == 0), stop=(do == NDO - 1))
                    osb = out_pool.tile([P, QC], f32, tag="osb",
                                        name=f"osb{qc}_{ec}")
                    nc.vector.tensor_copy(osb[:], pso[:])
                    nc.sync.dma_start(outT[es, qs], osb[:])
    nc.compile()
    return nc


def _get_nc():
    if "nc" not in _CACHE:
        _CACHE["nc"] = _build()
    return _CACHE["nc"]


def _in_maps(x, w_qkv, w_out):
    maps = []
    for c in range(8):
        b, g = c // 2, c % 2
        gs = slice(g * DG, (g + 1) * DG)
        maps.append({
            "xT": np.ascontiguousarray(x[b].T),
            "wq": np.ascontiguousarray(w_qkv[:, 0 * D:1 * D][:, gs]),
            "wk": np.ascontiguousarray(w_qkv[:, 1 * D:2 * D][:, gs]),
            "wv": np.ascontiguousarray(w_qkv[:, 2 * D:3 * D][:, gs]),
            "wo": np.ascontiguousarray(w_out[gs, :]),
            "eye8": _EYE8,
            "ind4": _IND4,
        })
    return maps


def kernel(x: np.ndarray, w_qkv: np.ndarray, w_out: np.ndarray) -> np.ndarray:
    x = np.asarray(x, dtype=np.float32)
    w_qkv = np.asarray(w_qkv, dtype=np.float32)
    w_out = np.asarray(w_out, dtype=np.float32)
    B = x.shape[0]
    nc = _get_nc()
    res = run_bass_kernel_spmd(nc, _in_maps(x, w_qkv, w_out),
                               core_ids=list(range(8)))
    out = np.empty((B, T, D), dtype=np.float32)
    for b in range(B):
        acc = res.results[2 * b]["outT"] + res.results[2 * b + 1]["outT"]
        out[b] = acc.T
    return out
